# revision 29
# baseline (speedup 1.0000x reference)
"""Multi-head attention (B=2, N=2048, C=1024, H=16) on 8 trn2 NeuronCores.

Tensor-parallel over heads: core c computes heads {2c, 2c+1} for both batch
elements and emits a partial output y_c = attn_out_c @ W_out[local rows]
(bf16 partials); the host sums the 8 partials and adds b_out.

v2: ScalarE-bound schedule. The exp stream (128 ACTIVATEs x ~1.15us =
~143us) is the kernel's hard floor (exp runs only on ScalarE at 1
elem/cycle/lane @1.2GHz; 16.8M S-elements/core = ~110us + 352cyc/instr
ramp). Everything else is scheduled to keep that stream fed:

  - ACT table preload: a dummy 1-element exp right at program start pulls
    the ~2.7us ACT_TABLE_LOAD off the critical exp path.
  - PE warmup: ~34 identity matmuls during the DMA head un-throttle the
    HAM clock gate (cold K=4/8 at 1.2GHz costs ~11us otherwise) so the
    first QKV chains run at 2.4GHz.
  - Head: weights are packed tensor-major (K|Q|V) so each is ONE DMA;
    x window 0 is spread across sync/vector/scalar queues; K and Q
    chains for batch-0 window-0 run first, S(0,0)/S(0,1) + exp are
    emitted before V window 0. First exp at ~8us (was ~22us).
  - All remaining QKV work (both batches) is placed on a static due-slot
    schedule: each ~1us part is injected at an explicit (q, kc) iteration
    of the attention loops, sized to the per-iteration PE slack
    (1.147us exp pace - 0.21 S pair - 0.43 PV pair ~= 0.5us). Batch-1's
    late windows (K3/V3/Q1-3) stream inside batch-1's own loop.
  - Output projections are deferred into a FIFO and drained by a credit
    pacer (never while a misc accumulator chain is open -- the 2-buffer
    misc pool would deadlock the in-order PE queue; never at kc 15).
    The final flush (after the last exp) spreads DMAs across
    scalar+sync+gpsimd.
  - Final q-chunk's reciprocal broadcast runs on the PE (ones-matmul)
    instead of GpSimd to shorten the serial tail.

Per-core pipeline details kept from v1: x^T loaded once into SBUF (bf16,
host pre-transposed, contiguous [128, 512] blocks); S^T = K @ Q^T as a
row-tiled pair (head h in rows h*64..) streaming concurrently; P^T =
exp(S^T/32) on ScalarE straight from PSUM ([128, 1024] ops); PV via
ones-augmented V (65th stationary column accumulates softmax
denominators); V transposed on the PE; normalization via DVE reciprocal +
GpSimd partition_broadcast; projection all-bf16. Never emit a consumer
before its producer: per-engine emission order IS program order.

absmax error ~5.2e-3 of the output scale vs the fp32 reference (bf16
operands; fp8 P/V was simulated and REJECTED: P e4m3 alone gives 2.7e-2).
"""
import os
import sys

sys.path.insert(0, "/opt/trn_rl_repo")

import ml_dtypes
import numpy as np

import concourse.bacc as bacc
import concourse.mybir as mybir
import concourse.tile as tile
from concourse import bass_utils
from concourse.masks import make_identity

F32 = mybir.dt.float32
BF16 = mybir.dt.bfloat16
NPBF16 = ml_dtypes.bfloat16

EMB = 1024
HEADS = 16
B = 2
SEQ = 2048
D = 64
NCORES = 8
HPC = HEADS // NCORES          # heads per core = 2
LD = HPC * D                   # local head dim = 128
TSEQ = B * SEQ                 # 4096
CC = EMB // 128                # contraction chunks = 8
SCALE = float(EMB) ** -0.5     # 1/32

QCH = 512                      # q chunk (free dim of S^T matmuls)
NQ = SEQ // QCH                # 4 q-chunks per batch
NK = SEQ // 128                # 16 k-chunks per batch
NW = TSEQ // 512               # 8 x windows (4 per batch)

# approximate PE costs (us) for the credit pacer
C_SPAIR = 0.22                 # S^T pair (row-tiled, concurrent)
C_PVPAIR = 0.44                # PV pair (2 serial N=512 matmuls)
C_EXP = 1.00                   # one [128,1024] ACTIVATE (queued cadence)
C_QKVP = 0.88                  # one qkv half-chain (4 N=512 matmuls)
C_PT = 1.15                    # V transpose unit (4 PE transposes)
C_PROJ = 0.22                  # one projection matmul


def _build():
    nc = bacc.Bacc("TRN2", target_bir_lowering=False, debug=False,
                   num_devices=NCORES)

    # x packed partition-major: xP[p, w, kc, c] = x-emb (kc*128+p) of
    # token (w*512+c). Each window is one fully-contiguous 1MB DMA.
    xP = nc.dram_tensor("xP", [128, NW, CC, 512], BF16,
                        kind="ExternalInput")
    # tensor-major: [k | q | v], each [128, CC*LD]
    wqkv = nc.dram_tensor("wqkv", [128, 3 * CC * LD], BF16,
                          kind="ExternalInput")
    bqkv = nc.dram_tensor("bqkv", [LD, 3], F32, kind="ExternalInput")
    wout = nc.dram_tensor("wout", [LD, EMB], BF16, kind="ExternalInput")
    y = nc.dram_tensor("y", [TSEQ // 128, 128, EMB], BF16,
                       kind="ExternalOutput")

    xP_c = xP.ap()
    wqkv_c = wqkv.ap()

    with tile.TileContext(nc) as tc:
        with (
            tc.tile_pool(name="persist", bufs=1) as persist,
            tc.tile_pool(name="vt", bufs=2) as vtp,
            tc.tile_pool(name="psb", bufs=8) as psb,
            tc.tile_pool(name="norm", bufs=3) as normp,
            tc.tile_pool(name="yout", bufs=10) as youtp,
            tc.tile_pool(name="ps_st", bufs=2, space="PSUM") as ps_st,
            tc.tile_pool(name="ps_pv", bufs=1, space="PSUM") as ps_pv,
            tc.tile_pool(name="ps_misc", bufs=2, space="PSUM") as ps_misc,
        ):
            # -------- head: ACT table preload + DMAs spread over queues
            dummy = persist.tile([1, 8], F32, tag="dummy")
            dummy_o = persist.tile([1, 8], F32, tag="dummy_o")
            nc.vector.memset(dummy[:], 0.0)
            nc.scalar.activation(dummy_o[:], dummy[:],
                                 mybir.ActivationFunctionType.Exp,
                                 scale=SCALE)

            bqkv_sb = persist.tile([LD, 3], F32, tag="bqkv")
            nc.scalar.dma_start(bqkv_sb[:], bqkv.ap())

            wall = persist.tile([128, 3 * CC * LD], BF16, tag="wall")
            WT = CC * LD  # 1024 cols per tensor
            # K weights first (one DMA), then Q on the scalar queue
            # (ScalarE is free until exp0 at ~8us), V on gpsimd
            nc.sync.dma_start(wall[:, 0:WT], wqkv_c[:, 0:WT])
            nc.scalar.dma_start(wall[:, WT:2 * WT], wqkv_c[:, WT:2 * WT])

            # per-window x tiles [128, CC, 512]; xfull[kc, w] slices one
            # contraction chunk.
            xw = [persist.tile([128, CC, 512], BF16, tag=f"xw{w}",
                               name=f"xw{w}") for w in range(NW)]
            xfull = {(kc, w): xw[w][:, kc]
                     for w in range(NW) for kc in range(CC)}

            # identity first: DVE memset + gpsimd affine_select BEFORE any
            # gpsimd DMA, so warmup matmuls can start at ~7.5us.
            ident = persist.tile([128, 128], BF16, tag="ident")
            nc.vector.memset(ident[:], 0.0)
            nc.gpsimd.affine_select(
                out=ident[:], in_=ident[:],
                compare_op=mybir.AluOpType.not_equal,
                fill=1.0, base=0, pattern=[[-1, 128]],
                channel_multiplier=1)

            # window 0 split across sync (kc 0-3, behind wk) and gpsimd
            # (kc 4-7, behind the affine_select); all land by ~9us.
            nc.sync.dma_start(xw[0][:, 0:4], xP_c[:, 0, 0:4])
            nc.gpsimd.dma_start(xw[0][:, 4:8], xP_c[:, 0, 4:8])
            nc.gpsimd.dma_start(wall[:, 2 * WT:3 * WT],
                                wqkv_c[:, 2 * WT:3 * WT])
            wout_sb = persist.tile([LD, EMB], BF16, tag="wout")
            nc.gpsimd.dma_start(wout_sb[:], wout.ap())

            # -------- PE warmup (HAM un-throttle): un-chained N=128
            # matmuls (each misc tile takes 4 writes to disjoint
            # quarters, so subtile deps leave them back-to-back) while
            # the head DMAs land; HAM's SHORT window then fires and the
            # real chains run at 2.4GHz.
            for i in range(3):
                wm = ps_misc.tile([128, 512], F32, tag="misc")
                for j in range(4):
                    nc.tensor.matmul(wm[:, j * 128:(j + 1) * 128],
                                     ident[:], ident[:],
                                     start=True, stop=True)

            # remaining x windows: ONE contiguous 1MB DMA each. sync
            # takes b0 wins 1-3 + b1 wins 6-7, gpsimd takes b1 wins 4-5.
            # ScalarE's queue stays clear for the exp stream.
            for w in (1, 2, 3):
                nc.sync.dma_start(xw[w][:], xP_c[:, w])
            for w in (4, 5):
                nc.gpsimd.dma_start(xw[w][:], xP_c[:, w])
            for w in (6, 7):
                nc.sync.dma_start(xw[w][:], xP_c[:, w])

            w_sb = {}
            for i, nm in enumerate(("k", "q", "v")):
                for kc in range(CC):
                    w_sb[nm, kc] = wall[:, i * WT + kc * LD:
                                        i * WT + (kc + 1) * LD]
            bias_sb = {nm: bqkv_sb[:, i:i + 1]
                       for i, nm in enumerate(("q", "k", "v"))}

            # persistent activations (per batch)
            QT = [persist.tile([LD, SEQ], BF16, tag=f"QT{b}",
                               name=f"QT{b}") for b in range(B)]
            KT = [persist.tile([LD, SEQ], BF16, tag=f"KT{b}", name=f"KT{b}")
                  for b in range(B)]
            outT = [persist.tile([LD, SEQ], BF16, tag=f"outT{b}",
                                 name=f"outT{b}") for b in range(B)]
            # vaug[b,kc][:, h, 0:64] = V^T chunk for head h; [:, h, 64] = 1
            # (ones-memsets are emitted later, after the critical K/Q
            # bias-adds, to keep the DVE queue clear in the head)
            vaug = {}
            for b in range(B):
                for kc in range(NK):
                    vaug[b, kc] = persist.tile([128, 2, 66], BF16,
                                               tag=f"vaug{b}_{kc}",
                                               name=f"vaug{b}_{kc}")

            # ---------------- building blocks --------------------------
            def qkv_parts(b, sc, nm):
                """Unit list [(fn, pe_cost, kind)] for one (batch,
                window, tensor) projection. kind: 'open' holds a misc
                accumulator until the matching 'close'; 'self' is
                self-contained."""
                s0 = sc * 512
                w = b * 4 + sc
                cell = {}

                def p1():
                    ps = ps_misc.tile([128, 512], F32, tag="misc")
                    cell["ps"] = ps
                    for kc in range(CC // 2):
                        nc.tensor.matmul(
                            ps[:], w_sb[nm, kc], xfull[kc, w][:],
                            start=(kc == 0), stop=False)

                def p2():
                    ps = cell["ps"]
                    for kc in range(CC // 2, CC):
                        nc.tensor.matmul(
                            ps[:], w_sb[nm, kc], xfull[kc, w][:],
                            start=False, stop=(kc == CC - 1))
                    if nm == "q":
                        nc.vector.tensor_scalar_add(
                            QT[b][:, s0:s0 + 512], ps[:], bias_sb["q"])
                    elif nm == "k":
                        nc.vector.tensor_scalar_add(
                            KT[b][:, s0:s0 + 512], ps[:], bias_sb["k"])
                    else:
                        vt = vtp.tile([128, 512], BF16, tag="vt")
                        nc.vector.tensor_scalar_add(vt[:], ps[:],
                                                    bias_sb["v"])
                        cell["vt"] = vt

                def pt_():
                    vt = cell["vt"]
                    pst4 = ps_misc.tile([128, 4, 2, D], BF16, tag="misc")
                    for j in range(4):
                        nc.tensor.transpose(
                            pst4[:, j], vt[:, j * 128:(j + 1) * 128],
                            ident[:])
                    for j in range(4):
                        nc.vector.tensor_copy(
                            vaug[b, sc * 4 + j][:, :, 0:D], pst4[:, j])

                if nm == "v":
                    return [(p1, C_QKVP, "open"), (p2, C_QKVP, "close"),
                            (pt_, C_PT, "pt")]
                return [(p1, C_QKVP, "open"), (p2, C_QKVP, "close")]

            pre_pts = {}

            def st_exp(b, q, kc):
                """S^T pair + exp for (batch, q-chunk, k-chunk)."""
                q0 = q * QCH
                st = ps_st.tile([128, 2 * QCH], F32, tag="st")
                k0 = kc * 128
                for h in range(HPC):
                    nc.tensor.matmul(
                        st[:, h * QCH:(h + 1) * QCH],
                        KT[b][h * D:(h + 1) * D, k0:k0 + 128],
                        QT[b][h * D:(h + 1) * D, q0:q0 + QCH],
                        start=True, stop=True)
                pt = psb.tile([128, 2 * QCH], BF16, tag="pt")
                nc.scalar.activation(pt[:], st[:],
                                     mybir.ActivationFunctionType.Exp,
                                     scale=SCALE)
                return pt

            pending = []

            def proj_unit(b, sc, n, eng=None, evict_eng=None):
                rt = b * (SEQ // 128) + sc
                ps = ps_misc.tile([128, 512], F32, tag="misc")
                nc.tensor.matmul(
                    ps[:], outT[b][:, sc * 128:(sc + 1) * 128],
                    wout_sb[:, n * 512:(n + 1) * 512],
                    start=True, stop=True)
                yt = youtp.tile([128, 512], BF16, tag="yt")
                if evict_eng is nc.scalar:
                    nc.scalar.copy(yt[:], ps[:])
                else:
                    nc.vector.tensor_copy(yt[:], ps[:])
                if eng is None:
                    eng = nc.gpsimd if (sc + n) % 2 else nc.sync
                eng.dma_start(
                    y.ap()[rt, :, n * 512:(n + 1) * 512], yt[:])

            # ---------------- attention phase ---------------------------
            def phase(b, due, final=False):
                """Attention for batch b. `due` maps (q, kc) -> unit list
                force-injected at that slot. Deferred projections drain
                via a credit pacer, never while a misc chain is open."""
                credit = 0.0
                guard = [0]

                def run_unit(u):
                    fn, cost, kind = u
                    if kind == "open":
                        guard[0] += 1
                    elif kind == "close":
                        guard[0] -= 1
                    fn()
                    return cost

                for q in range(NQ):
                    q0 = q * QCH
                    if q + 1 < NQ:
                        nxt = (b, q + 1)
                    elif b + 1 < B:
                        nxt = (b + 1, 0)
                    else:
                        nxt = None
                    pvs = [ps_pv.tile([D + 1, QCH], F32, tag=f"pv{h}",
                                      name=f"pv{h}") for h in range(HPC)]
                    for kc in range(NK):
                        pt = pre_pts.pop((b, q, kc), None)
                        if pt is None:
                            pt = st_exp(b, q, kc)
                        credit += C_EXP - C_PVPAIR
                        # forced due units (QKV chains / V transposes):
                        # run up to the last V-transpose (which must
                        # precede this slot's PV read of vaug) plus at
                        # least one unit BEFORE the S pair -- its ready
                        # matmuls absorb the S stall on exp(kc) and the
                        # pair's drain. The rest go after the PV pair.
                        dues = list(due.pop((q, kc), ()))
                        pt_idx = max((j for j, u in enumerate(dues)
                                      if u[2] == "pt"), default=-1)
                        ncut = max(pt_idx + 1, 1 if dues else 0)
                        for u in dues[:ncut]:
                            credit -= run_unit(u)
                        dues = dues[ncut:]
                        # two-deep S^T/exp lookahead
                        for ahead in (1, 2):
                            nkc = kc + ahead
                            if nkc < NK and (b, q, nkc) not in pre_pts:
                                pre_pts[b, q, nkc] = st_exp(b, q, nkc)
                                credit -= C_SPAIR
                        # cross-boundary: S(next,0) BEFORE the last PV
                        # pair -- it only WARs exp(q,14), so it streams
                        # during exp(q,15) and exp(next,0) queues with
                        # zero bubble.
                        if kc == NK - 1 and nxt is not None:
                            pre_pts[nxt[0], nxt[1], 0] = st_exp(
                                nxt[0], nxt[1], 0)
                            credit -= C_SPAIR
                        for h in range(HPC):
                            nc.tensor.matmul(
                                pvs[h][:],
                                vaug[b, kc][:, h, 0:D + 1],
                                pt[:, h * QCH:(h + 1) * QCH],
                                start=(kc == 0), stop=(kc == NK - 1))
                        for u in dues:
                            credit -= run_unit(u)
                        # paced projection drain. At kc 0/1 the PE is
                        # guaranteed to stall on the exp queue crossing
                        # the q-boundary, and in the final q-chunk the
                        # backlog must empty before the flush -- drain
                        # there regardless of credit (1/slot: more jams
                        # DVE and backs the misc pool into the PE).
                        force = kc <= 1 or (final and q == NQ - 1)
                        cap = 1 if (final and q == NQ - 1) else 2
                        if kc < 15 and not guard[0]:
                            npop = 0
                            while (pending and npop < cap
                                   and (credit >= C_PROJ or force)):
                                pending.pop(0)()
                                credit -= C_PROJ
                                npop += 1
                    # second cross-boundary pair (WARs exp(q,15))
                    if nxt is not None:
                        pre_pts[nxt[0], nxt[1], 1] = st_exp(
                            nxt[0], nxt[1], 1)
                        credit -= C_SPAIR
                    # normalize straight from PSUM: reciprocal reads the
                    # denominator row and the multiply reads the value
                    # rows in place (no eviction copies -- halves the
                    # per-boundary DVE chain; the pv banks are held
                    # until the muls, same release point as before).
                    rcss = []
                    for h in range(HPC):
                        ss = normp.tile([1, QCH], F32, tag="ss",
                                        name=f"ss{h}")
                        nc.vector.tensor_copy(ss[:], pvs[h][D:D + 1, :])
                        rcs = normp.tile([1, QCH], F32, tag="rcs",
                                         name=f"rcs{h}")
                        nc.vector.reciprocal_approx_fast(rcs[:], ss[:])
                        rcss.append(rcs)
                    # drain stale projections while DVE normalizes
                    # (their outT inputs are from older q-chunks: no PE
                    # wait; the PE is stalling here anyway, so ignore
                    # credit)
                    for _ in range(2):
                        if pending and not guard[0]:
                            pending.pop(0)()
                            credit -= C_PROJ
                    for h in range(HPC):
                        rb = normp.tile([D, QCH], F32, tag="rb")
                        nc.gpsimd.partition_broadcast(rb[:], rcss[h][:])
                        nc.vector.tensor_mul(
                            outT[b][h * D:(h + 1) * D, q0:q0 + QCH],
                            pvs[h][0:D, :], rb[:])
                    pending.extend(
                        (lambda b=b, sc=sc, n=n, eng=None, evict_eng=None:
                         proj_unit(b, sc, n, eng, evict_eng))
                        for sc in range(4 * q, 4 * q + 4)
                        for n in range(EMB // 512))

            # ---------------- emission program --------------------------
            # b0 win0: K then Q chains, then seed S(0,0)/S(0,1) so exp
            # starts ASAP; V win0 is forced at the first loop slot.
            for u in qkv_parts(0, 0, "k"):
                u[0]()
            for u in qkv_parts(0, 0, "q"):
                u[0]()
            pre_pts[0, 0, 0] = st_exp(0, 0, 0)
            pre_pts[0, 0, 1] = st_exp(0, 0, 1)
            # vaug ones columns (DVE queue is clear of critical adds now)
            for b in range(B):
                for kc in range(NK):
                    nc.vector.memset(vaug[b, kc][:, :, 64:65], 1.0)

            def put(due, q, kc, units):
                due.setdefault((q, kc), []).extend(units)

            # batch-0 loop schedule: own windows just-in-time, then
            # batch-1's early windows.
            due0 = {}
            put(due0, 0, 0, qkv_parts(0, 0, "v"))   # before PV(0)
            for w in (1, 2, 3):
                Kp = qkv_parts(0, w, "k")
                Vp = qkv_parts(0, w, "v")
                base = 4 * (w - 1)
                put(due0, 0, base + 1, [Kp[0]])
                put(due0, 0, base + 2, [Kp[1], Vp[0]])
                put(due0, 0, base + 3, [Vp[1]])
                put(due0, 0, base + 4, [Vp[2]])
            Q1 = qkv_parts(0, 1, "q")
            put(due0, 0, 13, [Q1[0]])
            put(due0, 0, 14, [Q1[1]])
            Q2 = qkv_parts(0, 2, "q")
            put(due0, 1, 0, [Q2[0]])
            put(due0, 1, 1, [Q2[1]])
            Q3 = qkv_parts(0, 3, "q")
            put(due0, 1, 4, [Q3[0]])
            put(due0, 1, 5, [Q3[1]])
            # batch-1 early windows spread over b0's q1-q3
            for q, kc, nm, w in [
                (1, 2, "k", 0), (1, 6, "q", 0), (1, 8, "v", 0),
                (1, 12, "k", 1),
                (2, 0, "v", 1), (2, 4, "k", 2), (2, 8, "v", 2),
                (3, 0, "q", 1), (3, 4, "q", 2),
            ]:
                for j, u in enumerate(qkv_parts(1, w, nm)):
                    put(due0, q, kc + j, [u])

            # batch-1 late windows stream inside batch-1's own loop.
            # K3 closes before the S(q0,12) lookahead at kc=10; V3's
            # transpose lands before PV(12); Q3 closes before the
            # q2-boundary pre-issue of S(q3,*).
            due1 = {}
            for q, kc, nm, w in [
                (0, 5, "k", 3), (0, 8, "v", 3), (0, 12, "q", 3),
            ]:
                for j, u in enumerate(qkv_parts(1, w, nm)):
                    put(due1, q, kc + j, [u])

            phase(0, due0)
            phase(1, due1, final=True)

            # flush remaining projections (b1 q3's 8 units); ScalarE is
            # free after the last exp, so it takes half the PSUM
            # evictions (halving the serial DVE chain) and a DMA share.
            engs = [nc.sync, nc.gpsimd, nc.scalar]
            for j, p in enumerate(pending):
                p(eng=engs[j % 3],
                  evict_eng=nc.scalar if j % 2 else nc.vector)
            del pending[:]

    nc.compile()
    return nc


_NC = None


def _get_nc():
    global _NC
    if _NC is None:
        _NC = _build()
    return _NC


def kernel(x, W_qkv, b_qkv, W_out, b_out):
    x = np.asarray(x, dtype=np.float32)
    W_qkv = np.asarray(W_qkv, dtype=np.float32)
    b_qkv = np.asarray(b_qkv, dtype=np.float32)
    W_out = np.asarray(W_out, dtype=np.float32)
    b_out = np.asarray(b_out, dtype=np.float32)

    nc = _get_nc()

    xT = x.reshape(TSEQ, EMB).T.astype(NPBF16).reshape(CC, 128, NW, 512)
    xPh = np.ascontiguousarray(xT.transpose(1, 2, 0, 3))  # [128,NW,CC,512]
    Wr = W_qkv.reshape(EMB, 3, HEADS, D)
    br = b_qkv.reshape(3, HEADS, D)

    in_maps = []
    for c in range(NCORES):
        h0, h1 = HPC * c, HPC * (c + 1)
        # weights tensor-major [k | q | v], each [128, CC*LD] chunk-major
        wt = np.stack(
            [Wr[:, i, h0:h1].reshape(CC, 128, LD) for i in (1, 0, 2)],
            axis=0)                       # [3(kqv), CC, 128, LD]
        wt = wt.transpose(2, 0, 1, 3).reshape(128, 3 * CC * LD)
        in_maps.append({
            "xP": xPh,
            "wqkv": np.ascontiguousarray(wt).astype(NPBF16),
            # bias columns stay (q, k, v) as in v1
            "bqkv": np.ascontiguousarray(
                np.stack([br[i, h0:h1].reshape(LD) for i in range(3)],
                         axis=1)),
            "wout": W_out[LD * c:LD * (c + 1)].astype(NPBF16),
        })

    res = bass_utils.run_bass_kernel_spmd(
        nc, in_maps, core_ids=list(range(NCORES)), trace=False)

    acc = np.zeros((TSEQ // 128, 128, EMB), dtype=np.float64)
    for c in range(NCORES):
        acc += res.results[c]["y"].astype(np.float64)
    out = (acc.reshape(TSEQ, EMB) + b_out).astype(np.float32)
    return out.reshape(B, SEQ, EMB)
